# revision 15
# baseline (speedup 1.0000x reference)
"""Bayesian linear layer (reparameterized sampling) on 8 Trainium2 NeuronCores.

Computes y = x @ (mu + softplus(rho) * eps_w)^T + (bias_mu + softplus(bias_rho) * eps_b)
with x [8192, 4096], weights [4096, 4096].

Strategy: column-parallel tensor parallelism. Each of the 8 cores owns a
512-wide slice of out_features: it materializes its weight slice
w_c = mu_c + softplus(rho_c) * eps_c on-chip (ACT exp/ln + DVE mul/add
in bf16), then computes y_c^T = w_c @ x^T on the TensorEngine (bf16
matmul, fp32 PSUM accumulation), fusing the bias add into the
PSUM->SBUF copy on the vector engine. x is replicated to all cores as
bf16 in [in_features, tokens] layout so the contraction dim lands on
partitions with no on-chip transposes. Outputs stay sharded ([512, 8192]
per core) and are concatenated/transposed on the host.

Startup-latency design. The steady state runs at the 216ns/MM N=512
bf16 stream rate, so all recoverable time lives in the first ~100us,
which is pinned to the ~360GB/s per-core HBM ceiling: the k-outer
prologue burns weights at ~220GB/s while x streams at ~130GB/s, so
there is no slack — every byte and every queue stall shows up as PE
idle. Specific measures:
- The activation-table pass greedily alternates exp_and_others /
  natural_log tables (1.3us ACT_TABLE_LOAD per switch). We blank those
  entries in the (index-preserving) cached table dict so the pass must
  pick natural_log_exp_and_others (400-point exp AND ln resident
  together) -> zero mid-kernel table churn, per-chunk softplus ladder.
- rho/mu ride one bit-interleaved 2-byte tensor (rho read back through
  a f16 bitcast) in 6 group DMAs on the scalar ring; 3 groups trigger
  upfront (wtmp bufs=3 so none of them parks on a buffer-reuse
  semaphore with ACT work queued behind it - the global DMA semaphore
  pool is ~10 deep and a waiting trigger blocks its whole queue).
- eps ships as fp8 e3m4 (4 mantissa bits; DVE auto-upconverts in the
  mul; ~1.1e-2 rel err vs the 2e-2 gate) - cuts 2MB off the critical
  startup stream - and rides the sync ring interleaved between x
  pieces, which balances the two rings' HBM pull toward the weight
  stream.
- The bias vectors ride one padded [P, 3, 64] f32 DMA issued first
  (the natural [P, 4] layout has 16-byte lines which take ~10us and
  head-of-line-block the ring; scheduler-hoisted bias ACT ops then
  stall the ladder), and the bias softplus runs after the ladder.
- ~28 dummy N=512 matmuls on memset tiles warm the PE_HAM clock gate
  (cold PE runs at 1.2GHz for its first ~3.4us of activity) while the
  first weight group streams in.
- The first two token-chunks accumulate k-outermost across 8 PSUM
  banks, ordered n-separated per x piece to match x arrival; x chunk 2
  is prefetched during the prologue (xpool bufs=3) so the steady loop
  starts the moment the prologue retires.
"""

import sys

for _p in ("/opt/trn_rl_repo",):
    if _p not in sys.path:
        sys.path.insert(0, _p)

import numpy as np
import ml_dtypes

IN_F = 4096
OUT_F = 4096
TOKENS = 8192
NCORES = 8
O_SH = OUT_F // NCORES  # 512 out-features per core

P = 128
NF = 512  # matmul free dim (one PSUM bank of fp32)
KG = 2    # k tiles per weight chunk
BPAD = 64  # bias padded free dim (keeps DMA lines at 256B)


def build_nc(in_f=IN_F, o_sh=O_SH, tokens=TOKENS):
    """Build the per-core Bass graph. All cores run the same graph (SPMD)."""
    import concourse.bass as bass  # noqa: F401
    import concourse.mybir as mybir
    from concourse import bacc, tile
    from concourse.hw_specs import get_activation_tables

    f32 = mybir.dt.float32
    bf16 = mybir.dt.bfloat16
    f16 = mybir.dt.float16
    f8e3 = mybir.dt.float8e3
    KO = in_f // P        # k tiles of 128
    MS = o_sh // P        # psum-partition (out-feature) subtiles
    NT = tokens // NF     # token chunks
    NSTREAM = min(2, NT)  # chunks computed k-outer while weights stream in
    EXP = mybir.ActivationFunctionType.Exp
    LN = mybir.ActivationFunctionType.Ln

    nc = bacc.Bacc(None, target_bir_lowering=False)

    # Force the combined exp+ln table (see module docstring).
    tables = get_activation_tables(nc.m.arch)
    comb = tables.get("natural_log_exp_and_others")
    if comb and EXP in comb and LN in comb:
        for name in ("exp_and_others", "natural_log", "exp_and_friends"):
            if name in tables:
                tables[name] = set()

    NKG = in_f // P // KG  # packed weight chunks (16)
    xT = nc.declare_dram_parameter("xT", [in_f, tokens], bf16, False)
    # rho(f16-bits) / mu interleaved on axis 2
    winT = nc.declare_dram_parameter("winT", [P, NKG, 2, KG, o_sh], bf16, False)
    epsT = nc.declare_dram_parameter("epsT", [P, NKG, KG, o_sh], f8e3, False)
    bvec = nc.declare_dram_parameter("bvec", [P, 3, BPAD], f32, False)
    out = nc.declare_dram_parameter("out", [o_sh, tokens], f32, True)

    # Partition-tiled views: row index r = ko*128 + p
    xT3 = xT[:].rearrange("(ko p) t -> p ko t", p=P)
    out3 = out[:].rearrange("(ms p) t -> p ms t", p=P)

    # weight DMA groups: tiny first group for chunk-0 latency; max 3
    # chunks per group bounds the staging SBUF at bufs=3
    WGROUPS = [(0, 1), (1, 4), (4, 7), (7, 10), (10, 13), (13, NKG)]
    if NKG <= 4:  # tiny sim sizes
        WGROUPS = [(0, NKG)]

    with tile.TileContext(nc) as tc:
        with (
            tc.tile_pool(name="warm", bufs=1) as warm_pool,
            tc.tile_pool(name="wpool", bufs=1) as wpool,
            tc.tile_pool(name="bias", bufs=1) as bias_pool,
            tc.tile_pool(name="xpool", bufs=3) as xpool,
            tc.tile_pool(name="opool", bufs=2) as opool,
            tc.tile_pool(name="psum", bufs=8, space="PSUM") as psum_pool,
        ):
            # ---- HAM warmup: dummy matmuls on zeroed tiles keep the PE
            # clock-gate at 8/8 while the first weight group streams in.
            wdum = warm_pool.tile([P, P], bf16, tag="wdum")
            nc.vector.memset(wdum[:], 0.0)
            xdum = warm_pool.tile([P, NF], bf16, tag="xdum")
            nc.vector.memset(xdum[:], 0.0)
            ps_warm = psum_pool.tile([P, NF], f32, tag="ps", name="ps_warm")
            NDUM = 28
            for i in range(NDUM):
                nc.tensor.matmul(ps_warm[:], wdum[:], xdum[:],
                                 start=(i == 0), stop=(i == NDUM - 1))

            # ---- bias DMA first on the sync ring: lands before any ACT
            # op could be scheduler-hoisted to wait on it.
            bv_t = bias_pool.tile([P, 3, BPAD], f32, tag="bv")
            nc.sync.dma_start(bv_t[:], bvec[:])
            b_sp = bias_pool.tile([P, MS], f32, tag="bsp")
            b_sb = bias_pool.tile([P, MS], f32, tag="bsb")

            # ---- x chunk loads (sync HWDGE ring): 4 k-slice 1MB pieces
            # per chunk so matmuls start after ~1MB instead of 4MB.
            NXP = 4
            KOP = KO // NXP

            def load_x_piece(n, q):
                xp = xpool.tile([P, KOP, NF], bf16, tag=f"x{q}",
                                name=f"x_{n}_{q}")
                nc.sync.dma_start(
                    xp[:], xT3[:, q * KOP: (q + 1) * KOP,
                               n * NF: (n + 1) * NF])
                return xp

            def load_x(n):
                return [load_x_piece(n, q) for q in range(NXP)]

            # ---- weight ladder pools
            wts = [None] * NKG
            with tc.tile_pool(name="spp", bufs=2) as spp, \
                 tc.tile_pool(name="spb1", bufs=1) as spb1, \
                 tc.tile_pool(name="wtmp", bufs=3) as wtmp:

                def trig_win(gi):
                    qb, qe = WGROUPS[gi]
                    nq = qe - qb
                    win_q = wtmp.tile([P, nq, 2, KG, o_sh], bf16, tag="win",
                                      name=f"win_{qb}")
                    nc.scalar.dma_start(win_q[:], winT[:][:, qb:qe, :, :, :])
                    return win_q

                def trig_eps(gi):
                    qb, qe = WGROUPS[gi]
                    nq = qe - qb
                    eps_q = wtmp.tile([P, nq, KG, o_sh], f8e3, tag="eps",
                                      name=f"eps_{qb}")
                    nc.sync.dma_start(eps_q[:], epsT[:][:, qb:qe, :, :])
                    return eps_q

                def act_chunk(win_q, eps_q, gi, kg):
                    qb, qe = WGROUPS[gi]
                    sp_b = spb1.tile([P, KG, o_sh], bf16, tag="spb")
                    nc.scalar.activation(
                        sp_b[:], win_q[:, kg - qb, 0].bitcast(f16), EXP)
                    sp_l = spp.tile([P, KG, o_sh], bf16, tag="spl")
                    nc.scalar.activation(sp_l[:], sp_b[:], LN, bias=1.0)
                    pr_t = spp.tile([P, KG, o_sh], bf16, tag="pr")
                    nc.vector.tensor_mul(pr_t[:], sp_l[:], eps_q[:, kg - qb])
                    w_t = wpool.tile([P, KG, o_sh], bf16, tag=f"wT{kg}")
                    nc.vector.tensor_add(w_t[:], pr_t[:], win_q[:, kg - qb, 1])
                    wts[kg] = w_t

                NW = len(WGROUPS)
                wgrp = [None] * NW
                egrp = [None] * NW
                # Global issue order tracks consumption deadlines: the
                # early fabric (~280GB/s) can't also carry weight groups
                # the PE won't touch for 30us. Scalar ring: win g0,g1
                # upfront, the rest staggered behind the act ladder.
                # Sync ring: bias, eps0, then x pieces with later eps
                # groups slotted between the piece pairs they precede.
                wgrp[0] = trig_win(0)
                if NW > 1:
                    wgrp[1] = trig_win(1)
                egrp[0] = trig_eps(0)
                xs = [[None] * NXP for _ in range(NSTREAM)]
                for n in range(NSTREAM):
                    xs[n][0] = load_x_piece(n, 0)
                if NW > 1:
                    egrp[1] = trig_eps(1)
                for q in range(1, NXP):
                    for n in range(NSTREAM):
                        xs[n][q] = load_x_piece(n, q)
                        if n == 0 and q + 1 < NW:  # eps g2..g5
                            egrp[q + 1] = trig_eps(q + 1)
                for gi in range(min(NXP + 1, NW), NW):  # eps tail (g5)
                    egrp[gi] = trig_eps(gi)
                # x chunk 2 prefetched (bufs=3): lands while the
                # prologue retires, so the steady loop starts gap-free.
                NPRE = min(NSTREAM + 1, NT)
                for n in range(NSTREAM, NPRE):
                    xs.append(load_x(n))

                for gi in range(NW):
                    qb, qe = WGROUPS[gi]
                    for kg in range(qb, qe):
                        act_chunk(wgrp[gi], egrp[gi], gi, kg)
                    if gi + 2 < NW:
                        wgrp[gi + 2] = trig_win(gi + 2)
                # bias softplus last: off the weight ladder's critical
                # path (same exp/ln table; needed only for the closes)
                nc.scalar.activation(b_sp[:], bv_t[:, 1, :MS], EXP)
                nc.scalar.activation(b_sp[:], b_sp[:], LN, bias=1.0)
                nc.vector.tensor_mul(b_sb[:], b_sp[:], bv_t[:, 2, :MS])
                nc.vector.tensor_add(b_sb[:], b_sb[:], bv_t[:, 0, :MS])

            def close_group(ps, ms, n):
                o_t = opool.tile([P, NF], f32, tag="o")
                nc.vector.tensor_scalar_add(o_t[:], ps[:], b_sb[:, ms: ms + 1])
                # scalar HWDGE ring is idle once the weight stream ends
                nc.scalar.dma_start(
                    out3[:, ms, n * NF: (n + 1) * NF], o_t[:]
                )

            # ---- prologue: NSTREAM chunks k-outer over 8 PSUM banks so
            # each weight chunk is consumed on arrival. MM order is
            # n-separated per x piece (all kos of piece q for n=0, then
            # n=1) so the first matmuls need only x piece (0,0).
            pss = [[psum_pool.tile([P, NF], f32, tag="ps",
                                   name=f"ps_s{n}_{ms}")
                    for ms in range(MS)]
                   for n in range(NSTREAM)]
            for q in range(NXP):
                for n in range(NSTREAM):
                    for kop in range(KOP):
                        ko = q * KOP + kop
                        w_sl = wts[ko // KG][:, ko % KG: ko % KG + 1, :]
                        for ms in range(MS):
                            nc.tensor.matmul(
                                pss[n][ms][:],
                                w_sl[:, :, ms * P: (ms + 1) * P],
                                xs[n][q][:, kop: kop + 1, :],
                                start=(ko == 0),
                                stop=(ko == KO - 1),
                            )
            for n in range(NSTREAM):
                for ms in range(MS):
                    close_group(pss[n][ms], ms, n)

            # ---- steady state: weights resident; k-innermost (PE-dense).
            for n in range(NSTREAM, NT):
                x_t = xs[n] if n < NPRE else load_x(n)
                for ms in range(MS):
                    ps = psum_pool.tile([P, NF], f32, tag="ps")
                    for ko in range(KO):
                        nc.tensor.matmul(
                            ps[:],
                            wts[ko // KG][:, ko % KG: ko % KG + 1,
                                          ms * P: (ms + 1) * P],
                            x_t[ko // KOP][:, ko % KOP: ko % KOP + 1, :],
                            start=(ko == 0),
                            stop=(ko == KO - 1),
                        )
                    close_group(ps, ms, n)

    nc.compile()
    return nc


def shard_inputs(x, weight_mu, weight_rho, bias_mu, bias_rho, eps_w, eps_b,
                 in_f=IN_F, o_sh=O_SH, tokens=TOKENS, ncores=NCORES):
    """Host-side layout + sharding: transpose to [in, out] / [in, tokens],
    bit-interleave rho(f16)/mu(bf16) into one 2-byte tensor; eps as e3m4."""
    bf16 = ml_dtypes.bfloat16
    f8e3 = ml_dtypes.float8_e3m4
    MS = o_sh // P
    KO = in_f // P
    NKG = KO // KG
    xT_bf = np.ascontiguousarray(np.asarray(x, dtype=np.float32).astype(bf16).T)
    mu_f = np.asarray(weight_mu, dtype=np.float32)
    rho_f = np.asarray(weight_rho, dtype=np.float32)
    eps_f = np.asarray(eps_w, dtype=np.float32)

    in_maps = []
    for c in range(ncores):
        sl = slice(c * o_sh, (c + 1) * o_sh)
        rho_u = rho_f[sl, :].T.astype(np.float16).view(np.uint16)
        mu_u = mu_f[sl, :].T.astype(bf16).view(np.uint16)
        stackd = np.stack([rho_u, mu_u], axis=1)  # [in_f, 2, o_sh]
        win = np.ascontiguousarray(
            stackd.reshape(NKG, KG, P, 2, o_sh).transpose(2, 0, 3, 1, 4)
        ).view(bf16)  # [P, NKG, 2, KG, o_sh]
        eps8 = np.ascontiguousarray(
            eps_f[sl, :].T.astype(f8e3)
            .reshape(NKG, KG, P, o_sh).transpose(2, 0, 1, 3)
        )  # [P, NKG, KG, o_sh]

        bv = np.zeros((P, 3, BPAD), dtype=np.float32)
        bv[:, 0, :MS] = np.asarray(bias_mu)[sl].reshape(MS, P).T
        bv[:, 1, :MS] = np.asarray(bias_rho)[sl].reshape(MS, P).T
        bv[:, 2, :MS] = np.asarray(eps_b)[sl].reshape(MS, P).T

        in_maps.append({
            "xT": xT_bf,
            "winT": win,
            "epsT": eps8,
            "bvec": bv,
        })
    return in_maps


_NC_CACHE = {}


def _get_nc():
    if "nc" not in _NC_CACHE:
        _NC_CACHE["nc"] = build_nc()
    return _NC_CACHE["nc"]


def kernel(x, weight_mu, weight_rho, bias_mu, bias_rho, eps_w, eps_b):
    from concourse import bass_utils

    nc = _get_nc()
    in_maps = shard_inputs(x, weight_mu, weight_rho, bias_mu, bias_rho, eps_w, eps_b)
    res = bass_utils.run_bass_kernel_spmd(nc, in_maps, core_ids=list(range(NCORES)))
    yT = np.concatenate([res.results[c]["out"] for c in range(NCORES)], axis=0)
    return np.ascontiguousarray(yT.T)
